# revision 9
# baseline (speedup 1.0000x reference)
"""Trainium2 Bass kernel for nn_DecoderRNN: 64-step 2-layer tanh RNN + per-step FC.

Sharding (8 cores, no collectives):
  - 2-way data parallel over batch (cores 0-3: rows 0:128, cores 4-7: rows 128:256).
    Each group of 4 cores redundantly computes its batch-half's RNN (the RNN is
    1/3 of total FLOPs; replication buys full M=128 PE utilization).
  - 4-way tensor parallel over the fc output dim (O=8192 -> 2048 per core).

Per-core compute, per step t (B=128, H=1024, O_slice=2048):
  - State kept transposed ("g" form, [H, B]): g tiles are the matmul stationary
    operand (lhsT), weights stream as the moving operand at N=512.
  - RNN matmuls in float32r (full-rate 4-byte mode, ~11-bit mantissa on HW),
    accumulating x@W_ihT + h@W_hhT + bias into PSUM; bias enters via a K=1
    matmul against a ones row. tanh on the scalar engine, fp32.
  - Layer outputs ([B, H] in PSUM) are transposed back to g form on the tensor
    engine (8x 128x128 transposes per layer output).
  - FC in bf16 (weights resident in SBUF as bf16; top-layer g also copied to
    bf16), logits copied PSUM->SBUF->HBM per step.
"""
import sys

sys.path.insert(0, "/opt/trn_rl_repo")

from contextlib import ExitStack

import numpy as np
import ml_dtypes

import concourse.bass as bass
import concourse.tile as tile
from concourse import bacc, mybir
from concourse.bass_utils import run_bass_kernel_spmd

H = 1024
O = 8192
L = 2
T = 64
B = 256
N_CORES = 8
BG = B // 2          # batch rows per core (2-way DP)
OS = O // 4          # fc output slice per core (4-way TP)
KT = H // 128        # 8 k-tiles per 1024 contraction
F32 = mybir.dt.float32
F32R = mybir.dt.float32r
BF16 = mybir.dt.bfloat16

_cached = {}


def _build_program(n_steps: int, n_reps: int = 1):
    nc = bacc.Bacc("TRN2", target_bir_lowering=False, debug=False, num_devices=N_CORES)

    # --- DRAM parameters (per-core shards, host-prepared layouts) ---
    # RNN weights, transposed+tiled on host: [p][k][n] with W.T[k*128+p, n]
    w_ih0 = nc.declare_dram_parameter("w_ih0", [128, KT, H], F32R, isOutput=False)
    w_hh0 = nc.declare_dram_parameter("w_hh0", [128, KT, H], F32R, isOutput=False)
    w_ih1 = nc.declare_dram_parameter("w_ih1", [128, KT, H], F32R, isOutput=False)
    w_hh1 = nc.declare_dram_parameter("w_hh1", [128, KT, H], F32R, isOutput=False)
    w_fc = nc.declare_dram_parameter("w_fc", [128, KT, OS], BF16, isOutput=False)
    # initial state, g form: [p][k][b] = state[b, k*128+p]
    g_x0 = nc.declare_dram_parameter("g_x0", [128, KT, BG], F32R, isOutput=False)
    g_h0i = nc.declare_dram_parameter("g_h0i", [128, KT, BG], F32R, isOutput=False)
    g_h1i = nc.declare_dram_parameter("g_h1i", [128, KT, BG], F32R, isOutput=False)
    # bias rows (b_ih + b_hh per layer), fc bias slice, ones rows, identity
    b0d = nc.declare_dram_parameter("b0", [1, H], F32R, isOutput=False)
    b1d = nc.declare_dram_parameter("b1", [1, H], F32R, isOutput=False)
    fcbd = nc.declare_dram_parameter("fcb", [1, OS], BF16, isOutput=False)
    onesd = nc.declare_dram_parameter("ones", [1, 128], F32R, isOutput=False)
    onesbd = nc.declare_dram_parameter("onesb", [1, 128], BF16, isOutput=False)
    identd = nc.declare_dram_parameter("ident", [128, 128], F32, isOutput=False)

    out_d = nc.declare_dram_parameter("out", [n_steps, 128, OS], F32, isOutput=True)

    with tile.TileContext(nc) as tc, ExitStack() as ctx:
        wpool = ctx.enter_context(tc.tile_pool(name="w", bufs=1))
        cpool = ctx.enter_context(tc.tile_pool(name="c", bufs=1))
        g32p = ctx.enter_context(tc.tile_pool(name="g32", bufs=3))
        gbfp = ctx.enter_context(tc.tile_pool(name="gbf", bufs=2))
        hp = ctx.enter_context(tc.tile_pool(name="h", bufs=2))
        logp = ctx.enter_context(tc.tile_pool(name="log", bufs=2))
        rnn_ps = ctx.enter_context(tc.tile_pool(name="rnnps", bufs=2, space="PSUM"))
        tr_ps = ctx.enter_context(tc.tile_pool(name="trps", bufs=2, space="PSUM"))
        fc_ps = ctx.enter_context(tc.tile_pool(name="fcps", bufs=1, space="PSUM"))

        # --- preamble: load weights/constants ---
        w = {}
        for name, dram in [("ih0", w_ih0), ("hh0", w_hh0), ("ih1", w_ih1), ("hh1", w_hh1)]:
            t_ = wpool.tile([128, KT, H], F32R, tag=f"w{name}")
            nc.sync.dma_start(t_[:], dram[:])
            w[name] = t_
        wfc = wpool.tile([128, KT, OS], BF16, tag="wfc")
        nc.sync.dma_start(wfc[:], w_fc[:])

        b0 = cpool.tile([1, H], F32R, tag="b0")
        b1 = cpool.tile([1, H], F32R, tag="b1")
        fcb = cpool.tile([1, OS], BF16, tag="fcb")
        ones = cpool.tile([1, 128], F32R, tag="ones")
        onesb = cpool.tile([1, 128], BF16, tag="onesb")
        ident = cpool.tile([128, 128], F32, tag="ident")
        for t_, d_ in [(b0, b0d), (b1, b1d), (fcb, fcbd), (ones, onesd),
                       (onesb, onesbd), (ident, identd)]:
            nc.sync.dma_start(t_[:], d_[:])

        # --- initial state ---
        g_x = g32p.tile([128, KT, BG], F32R, tag="g32")
        g_h0 = g32p.tile([128, KT, BG], F32R, tag="g32")
        g_h1 = g32p.tile([128, KT, BG], F32R, tag="g32")
        nc.sync.dma_start(g_x[:], g_x0[:])
        nc.sync.dma_start(g_h0[:], g_h0i[:])
        nc.sync.dma_start(g_h1[:], g_h1i[:])

        def rnn_layer(g_in, g_h, w_in, w_h, brow):
            """tanh(in @ W_ihT + h @ W_hhT + b) -> h_sbuf [128(B), H] fp32."""
            ps = rnn_ps.tile([128, H], F32, tag="rnnps")
            for nck in range(2):
                nsl = bass.ts(nck, 512)
                nc.tensor.matmul(ps[:, nsl], ones[:, :],
                                 brow[:, nsl], start=True, stop=False)
                for k in range(KT):
                    nc.tensor.matmul(ps[:, nsl], g_in[:, k, :],
                                     w_in[:, k, nsl],
                                     start=False, stop=False)
                for k in range(KT):
                    nc.tensor.matmul(ps[:, nsl], g_h[:, k, :],
                                     w_h[:, k, nsl],
                                     start=False, stop=(k == KT - 1))
            h_sb = hp.tile([128, H], F32, tag="h")
            nc.scalar.activation(h_sb[:], ps[:], mybir.ActivationFunctionType.Tanh)
            return h_sb

        def to_g(h_sb, want_bf16):
            """Transpose [B, H] -> g form [H(p), B], fp32 (+ optional bf16 copy)."""
            g32 = g32p.tile([128, KT, BG], F32R, tag="g32")
            gbf = gbfp.tile([128, KT, BG], BF16, tag="gbf", name="gbf") if want_bf16 else None
            for k in range(KT):
                pt = tr_ps.tile([128, 128], F32, tag="trps")
                nc.tensor.transpose(pt[:], h_sb[:, bass.ts(k, 128)], ident[:])
                nc.vector.tensor_copy(g32[:, k, :], pt[:])
                if want_bf16:
                    nc.vector.tensor_copy(gbf[:, k, :], pt[:])
            return g32, gbf

        for t in range(n_steps * n_reps):
            t = t % n_steps
            h0_sb = rnn_layer(g_x, g_h0, w["ih0"], w["hh0"], b0)
            g_h0, _ = to_g(h0_sb, want_bf16=False)
            h1_sb = rnn_layer(g_h0, g_h1, w["ih1"], w["hh1"], b1)
            g_h1, g_h1bf = to_g(h1_sb, want_bf16=True)
            g_x = g_h1

            # FC: logits[B, OS] = h1 @ fc_W_slice.T + fc_b_slice   (bf16)
            # two [128, 1024] halves to keep PSUM at 2 banks
            for half in range(2):
                ps = fc_ps.tile([128, OS // 2], F32, tag="fcps", name="fps")
                for nck in range(2):
                    fsl = bass.ts(half * 2 + nck, 512)   # slice into wfc/fcb
                    nsl = bass.ts(nck, 512)              # slice into ps
                    nc.tensor.matmul(ps[:, nsl], onesb[:, :], fcb[:, fsl],
                                     start=True, stop=False)
                    for k in range(KT):
                        nc.tensor.matmul(ps[:, nsl], g_h1bf[:, k, :], wfc[:, k, fsl],
                                         start=False, stop=(k == KT - 1))
                lsb = logp.tile([128, OS // 2], F32, tag="log", name="lsb")
                nc.vector.tensor_copy(lsb[:], ps[:])
                nc.sync.dma_start(out_d[t][:, bass.ts(half, OS // 2)], lsb[:])

    nc.finalize()
    return nc


def _prep_inputs(x, hidden, W_ih, W_hh, b_ih, b_hh, fc_W, fc_b, n_steps):
    """Build the 8 per-core input maps (host-side transposes / dtype prep)."""
    def gform(a):  # [BG, H] -> [128, KT, BG]: out[p, k, b] = a[b, k*128+p]
        return np.ascontiguousarray(
            a.T.reshape(KT, 128, BG).transpose(1, 0, 2)).astype(np.float32)

    def wform(Wmat):  # [H_out, H_in] -> [128, KT, H_out] of W.T
        return np.ascontiguousarray(
            Wmat.T.reshape(KT, 128, Wmat.shape[0]).transpose(1, 0, 2)).astype(np.float32)

    ident = np.eye(128, dtype=np.float32)
    ones = np.ones((1, 128), np.float32)
    onesb = np.ones((1, 128), ml_dtypes.bfloat16)
    b0 = (b_ih[0] + b_hh[0]).astype(np.float32).reshape(1, H)
    b1 = (b_ih[1] + b_hh[1]).astype(np.float32).reshape(1, H)
    wi0, wh0 = wform(W_ih[0]), wform(W_hh[0])
    wi1, wh1 = wform(W_ih[1]), wform(W_hh[1])

    in_maps = []
    for c in range(N_CORES):
        bg, j = c // 4, c % 4
        bsl = slice(bg * BG, (bg + 1) * BG)
        osl = slice(j * OS, (j + 1) * OS)
        wfc = np.ascontiguousarray(
            fc_W[osl].T.reshape(KT, 128, OS).transpose(1, 0, 2)).astype(ml_dtypes.bfloat16)
        in_maps.append({
            "w_ih0": wi0, "w_hh0": wh0, "w_ih1": wi1, "w_hh1": wh1,
            "w_fc": wfc,
            "g_x0": gform(x[0, bsl]),
            "g_h0i": gform(hidden[0, bsl]),
            "g_h1i": gform(hidden[1, bsl]),
            "b0": b0, "b1": b1,
            "fcb": fc_b[osl].astype(ml_dtypes.bfloat16).reshape(1, OS),
            "ones": ones, "onesb": onesb, "ident": ident,
        })
    return in_maps


def kernel(x, hidden, embedded, W_ih, W_hh, b_ih, b_hh, fc_W, fc_b,
           _trace=False, _trace_kwargs=None):
    n_steps = embedded.shape[0]
    key = n_steps
    if key not in _cached:
        _cached[key] = _build_program(n_steps)
    nc = _cached[key]

    in_maps = _prep_inputs(np.asarray(x), np.asarray(hidden), np.asarray(W_ih),
                           np.asarray(W_hh), np.asarray(b_ih), np.asarray(b_hh),
                           np.asarray(fc_W), np.asarray(fc_b), n_steps)
    core_ids = list(range(N_CORES))
    res = run_bass_kernel_spmd(nc, in_maps, core_ids, trace=_trace,
                               **(_trace_kwargs or {}))

    out = np.empty((n_steps, 1, B, O), np.float32)
    for c in range(N_CORES):
        bg, j = c // 4, c % 4
        out[:, 0, bg * BG:(bg + 1) * BG, j * OS:(j + 1) * OS] = res.results[c]["out"]
    if _trace:
        kernel.last_results = res
    return out


# revision 10
# speedup vs baseline: 2.0789x; 2.0789x over previous
"""Trainium2 Bass kernel for nn_DecoderRNN: 64-step 2-layer tanh RNN + per-step FC.

Sharding (8 cores, no collectives):
  - 2-way data parallel over batch (cores 0-3: rows 0:128, cores 4-7: rows 128:256).
    Each group of 4 cores redundantly computes its batch-half's RNN (the RNN is
    1/3 of total FLOPs; replication buys full M=128 PE utilization).
  - 4-way tensor parallel over the fc output dim (O=8192 -> 2048 per core).

Per-core compute, per step t (B=128, H=1024, O_slice=2048):
  - State kept transposed ("g" form, [H, B]): g tiles are the matmul stationary
    operand (lhsT), weights stream as the moving operand at N=512.
  - RNN matmuls in float32r (full-rate 4-byte mode, ~11-bit mantissa on HW),
    accumulating x@W_ihT + h@W_hhT + bias into PSUM; bias enters via a K=1
    matmul against a ones row. tanh on the scalar engine, fp32.
  - Layer outputs ([B, H] in PSUM) are transposed back to g form on the tensor
    engine (8x 128x128 transposes per layer output).
  - FC in bf16 (weights resident in SBUF as bf16; top-layer g also copied to
    bf16), logits copied PSUM->SBUF->HBM per step.
"""
import sys

sys.path.insert(0, "/opt/trn_rl_repo")

from contextlib import ExitStack

import numpy as np
import ml_dtypes

import concourse.bass as bass
import concourse.tile as tile
from concourse import bacc, mybir
from concourse.bass_utils import run_bass_kernel_spmd

H = 1024
O = 8192
L = 2
T = 64
B = 256
N_CORES = 8
BG = B // 2          # batch rows per core (2-way DP)
OS = O // 4          # fc output slice per core (4-way TP)
KT = H // 128        # 8 k-tiles per 1024 contraction
F32 = mybir.dt.float32
F32R = mybir.dt.float32r
BF16 = mybir.dt.bfloat16

_cached = {}


def _build_program(n_steps: int, n_reps: int = 1):
    nc = bacc.Bacc("TRN2", target_bir_lowering=False, debug=False, num_devices=N_CORES)

    # --- DRAM parameters (per-core shards, host-prepared layouts) ---
    # RNN weights, transposed+tiled on host: [p][k][n] with W.T[k*128+p, n]
    w_ih0 = nc.declare_dram_parameter("w_ih0", [128, KT, H], F32R, isOutput=False)
    w_hh0 = nc.declare_dram_parameter("w_hh0", [128, KT, H], F32R, isOutput=False)
    w_ih1 = nc.declare_dram_parameter("w_ih1", [128, KT, H], F32R, isOutput=False)
    w_hh1 = nc.declare_dram_parameter("w_hh1", [128, KT, H], F32R, isOutput=False)
    w_fc = nc.declare_dram_parameter("w_fc", [128, KT, OS], BF16, isOutput=False)
    # initial state, g form: [p][k][b] = state[b, k*128+p]
    g_x0 = nc.declare_dram_parameter("g_x0", [128, KT, BG], F32R, isOutput=False)
    g_h0i = nc.declare_dram_parameter("g_h0i", [128, KT, BG], F32R, isOutput=False)
    g_h1i = nc.declare_dram_parameter("g_h1i", [128, KT, BG], F32R, isOutput=False)
    # bias rows (b_ih + b_hh per layer), fc bias slice, ones rows, identity
    b0d = nc.declare_dram_parameter("b0", [1, H], F32R, isOutput=False)
    b1d = nc.declare_dram_parameter("b1", [1, H], F32R, isOutput=False)
    fcbd = nc.declare_dram_parameter("fcb", [1, OS], BF16, isOutput=False)
    onesd = nc.declare_dram_parameter("ones", [1, 128], F32R, isOutput=False)
    onesbd = nc.declare_dram_parameter("onesb", [1, 128], BF16, isOutput=False)
    identd = nc.declare_dram_parameter("ident", [128, 128], F32, isOutput=False)

    out_d = nc.declare_dram_parameter("out", [n_steps, 128, OS], F32, isOutput=True)

    with tile.TileContext(nc) as tc, ExitStack() as ctx:
        wpool = ctx.enter_context(tc.tile_pool(name="w", bufs=1))
        cpool = ctx.enter_context(tc.tile_pool(name="c", bufs=1))
        g32p = ctx.enter_context(tc.tile_pool(name="g32", bufs=3))
        gbfp = ctx.enter_context(tc.tile_pool(name="gbf", bufs=2))
        hp = ctx.enter_context(tc.tile_pool(name="h", bufs=2))
        logp = ctx.enter_context(tc.tile_pool(name="log", bufs=2))
        rnn_ps = ctx.enter_context(tc.tile_pool(name="rnnps", bufs=1, space="PSUM"))
        tr_ps = ctx.enter_context(tc.tile_pool(name="trps", bufs=2, space="PSUM"))
        fc_ps = ctx.enter_context(tc.tile_pool(name="fcps", bufs=1, space="PSUM"))

        # --- preamble: load weights/constants ---
        w = {}
        for name, dram in [("ih0", w_ih0), ("hh0", w_hh0), ("ih1", w_ih1), ("hh1", w_hh1)]:
            t_ = wpool.tile([128, KT, H], F32R, tag=f"w{name}")
            nc.sync.dma_start(t_[:], dram[:])
            w[name] = t_
        wfc = wpool.tile([128, KT, OS], BF16, tag="wfc")
        nc.sync.dma_start(wfc[:], w_fc[:])

        b0 = cpool.tile([1, H], F32R, tag="b0")
        b1 = cpool.tile([1, H], F32R, tag="b1")
        fcb = cpool.tile([1, OS], BF16, tag="fcb")
        ones = cpool.tile([1, 128], F32R, tag="ones")
        onesb = cpool.tile([1, 128], BF16, tag="onesb")
        ident = cpool.tile([128, 128], F32, tag="ident")
        for t_, d_ in [(b0, b0d), (b1, b1d), (fcb, fcbd), (ones, onesd),
                       (onesb, onesbd), (ident, identd)]:
            nc.sync.dma_start(t_[:], d_[:])

        # --- initial state ---
        g_x = g32p.tile([128, KT, BG], F32R, tag="g32")
        g_h0 = g32p.tile([128, KT, BG], F32R, tag="g32")
        g_h1 = g32p.tile([128, KT, BG], F32R, tag="g32")
        nc.sync.dma_start(g_x[:], g_x0[:])
        nc.sync.dma_start(g_h0[:], g_h0i[:])
        nc.sync.dma_start(g_h1[:], g_h1i[:])

        def rnn_layer(g_in, g_h, w_in, w_h, brow):
            """tanh(in @ W_ihT + h @ W_hhT + b) -> h_sbuf [128(B), H] fp32."""
            ps = rnn_ps.tile([128, H], F32, tag="rnnps")
            for nck in range(2):
                nsl = bass.ts(nck, 512)
                nc.tensor.matmul(ps[:, nsl], ones[:, :],
                                 brow[:, nsl], start=True, stop=False)
                for k in range(KT):
                    nc.tensor.matmul(ps[:, nsl], g_in[:, k, :],
                                     w_in[:, k, nsl],
                                     start=False, stop=False)
                for k in range(KT):
                    nc.tensor.matmul(ps[:, nsl], g_h[:, k, :],
                                     w_h[:, k, nsl],
                                     start=False, stop=(k == KT - 1))
            h_sb = hp.tile([128, H], F32, tag="h")
            nc.scalar.activation(h_sb[:], ps[:], mybir.ActivationFunctionType.Tanh)
            return h_sb

        def to_g(h_sb, want_bf16):
            """Transpose [B, H] -> g form [H(p), B], fp32 (+ optional bf16 copy)."""
            g32 = g32p.tile([128, KT, BG], F32R, tag="g32")
            gbf = gbfp.tile([128, KT, BG], BF16, tag="gbf", name="gbf") if want_bf16 else None
            for k in range(KT):
                pt = tr_ps.tile([128, 128], F32, tag="trps")
                nc.tensor.transpose(pt[:], h_sb[:, bass.ts(k, 128)], ident[:])
                nc.vector.tensor_copy(g32[:, k, :], pt[:])
                if want_bf16:
                    nc.vector.tensor_copy(gbf[:, k, :], pt[:])
            return g32, gbf

        for t in range(n_steps * n_reps):
            t = t % n_steps
            h0_sb = rnn_layer(g_x, g_h0, w["ih0"], w["hh0"], b0)
            g_h0, _ = to_g(h0_sb, want_bf16=False)
            h1_sb = rnn_layer(g_h0, g_h1, w["ih1"], w["hh1"], b1)
            g_h1, g_h1bf = to_g(h1_sb, want_bf16=True)
            g_x = g_h1

            # FC: logits[B, OS] = h1 @ fc_W_slice.T + fc_b_slice   (bf16)
            # two [128, 1024] halves to keep PSUM at 2 banks
            for half in range(2):
                ps = fc_ps.tile([128, OS // 2], F32, tag="fcps", name="fps")
                for nck in range(2):
                    fsl = bass.ts(half * 2 + nck, 512)   # slice into wfc/fcb
                    nsl = bass.ts(nck, 512)              # slice into ps
                    nc.tensor.matmul(ps[:, nsl], onesb[:, :], fcb[:, fsl],
                                     start=True, stop=False)
                    for k in range(KT):
                        nc.tensor.matmul(ps[:, nsl], g_h1bf[:, k, :], wfc[:, k, fsl],
                                         start=False, stop=(k == KT - 1))
                lsb = logp.tile([128, OS // 2], F32, tag="log", name="lsb")
                nc.vector.tensor_copy(lsb[:], ps[:])
                nc.sync.dma_start(out_d[t][:, bass.ts(half, OS // 2)], lsb[:])

    nc.finalize()
    return nc


def _prep_inputs(x, hidden, W_ih, W_hh, b_ih, b_hh, fc_W, fc_b, n_steps):
    """Build the 8 per-core input maps (host-side transposes / dtype prep)."""
    def gform(a):  # [BG, H] -> [128, KT, BG]: out[p, k, b] = a[b, k*128+p]
        return np.ascontiguousarray(
            a.T.reshape(KT, 128, BG).transpose(1, 0, 2)).astype(np.float32)

    def wform(Wmat):  # [H_out, H_in] -> [128, KT, H_out] of W.T
        return np.ascontiguousarray(
            Wmat.T.reshape(KT, 128, Wmat.shape[0]).transpose(1, 0, 2)).astype(np.float32)

    ident = np.eye(128, dtype=np.float32)
    ones = np.ones((1, 128), np.float32)
    onesb = np.ones((1, 128), ml_dtypes.bfloat16)
    b0 = (b_ih[0] + b_hh[0]).astype(np.float32).reshape(1, H)
    b1 = (b_ih[1] + b_hh[1]).astype(np.float32).reshape(1, H)
    wi0, wh0 = wform(W_ih[0]), wform(W_hh[0])
    wi1, wh1 = wform(W_ih[1]), wform(W_hh[1])

    in_maps = []
    for c in range(N_CORES):
        bg, j = c // 4, c % 4
        bsl = slice(bg * BG, (bg + 1) * BG)
        osl = slice(j * OS, (j + 1) * OS)
        wfc = np.ascontiguousarray(
            fc_W[osl].T.reshape(KT, 128, OS).transpose(1, 0, 2)).astype(ml_dtypes.bfloat16)
        in_maps.append({
            "w_ih0": wi0, "w_hh0": wh0, "w_ih1": wi1, "w_hh1": wh1,
            "w_fc": wfc,
            "g_x0": gform(x[0, bsl]),
            "g_h0i": gform(hidden[0, bsl]),
            "g_h1i": gform(hidden[1, bsl]),
            "b0": b0, "b1": b1,
            "fcb": fc_b[osl].astype(ml_dtypes.bfloat16).reshape(1, OS),
            "ones": ones, "onesb": onesb, "ident": ident,
        })
    return in_maps


def kernel(x, hidden, embedded, W_ih, W_hh, b_ih, b_hh, fc_W, fc_b,
           _trace=False, _trace_kwargs=None):
    n_steps = embedded.shape[0]
    key = n_steps
    if key not in _cached:
        _cached[key] = _build_program(n_steps)
    nc = _cached[key]

    in_maps = _prep_inputs(np.asarray(x), np.asarray(hidden), np.asarray(W_ih),
                           np.asarray(W_hh), np.asarray(b_ih), np.asarray(b_hh),
                           np.asarray(fc_W), np.asarray(fc_b), n_steps)
    core_ids = list(range(N_CORES))
    res = run_bass_kernel_spmd(nc, in_maps, core_ids, trace=_trace,
                               **(_trace_kwargs or {}))

    out = np.empty((n_steps, 1, B, O), np.float32)
    for c in range(N_CORES):
        bg, j = c // 4, c % 4
        out[:, 0, bg * BG:(bg + 1) * BG, j * OS:(j + 1) * OS] = res.results[c]["out"]
    if _trace:
        kernel.last_results = res
    return out
